# revision 13
# baseline (speedup 1.0000x reference)
"""Trainium2 Bass kernel for causal self-attention with T5 relative position bias.

Problem (hardcoded): B=4, T=2048, C=1024, H=16, D=64, NUM_BUCKETS=32, MAX_DISTANCE=128.
Sharding over 8 cores: core c -> (batch b=c//2, head-group hg=c%2 of 8 heads).
Each core computes qkv projection for its heads, causal attention, and a partial
output projection (its heads' rows of W_proj); host sums the two partials per batch.

On-chip layout notes:
  - x, q, k are kept transposed ([C, T]-style, channel on partitions) so every
    matmul contracts over the partition dim with no on-chip transposes.
  - Attention logits are computed transposed: S_T[tk, tq] = k_h^T q_h (K=64).
  - Softmax skips max-subtraction (logits ~ N(0,1); exp <= e^7 fits fp16 easily).
  - The T5 bias + causal mask are folded into one fp16 Toeplitz table per head:
    expAm[p, x] = exp(bias[d]) * (d >= 0), d = x - p - 384.  P = exp(S/8) * expAm.
    The [128, 2432] table is expanded on-device from a 2559-float vector by a
    DMA whose DRAM-side access pattern has a -1 partition step.
  - Row sums come free from a ones-column appended to V (AV matmul M=65);
    normalization multiplies by a PE-broadcast reciprocal row.
"""

import sys

sys.path.insert(0, "/opt/trn_rl_repo")

import math

import numpy as np

import concourse.bacc as bacc
import concourse.bass as bass
import concourse.mybir as mybir
import concourse.tile as tile
from concourse import bass_utils

B, T, C = 4, 2048, 1024
H, D = 16, 64
NUM_BUCKETS, MAX_DISTANCE = 32, 128
HL = 8  # local heads per core
CL = HL * D  # 512 local channels
NCORES = 8

FP16 = mybir.dt.float16
FP32 = mybir.dt.float32

# expAm table geometry: slice start s = (tq0 - tk0) + 384 in [0, 1920], width 512
EA_W = 2432  # 1920 + 512
EA_VEC = EA_W + 127  # 2559: w[j] = exp(bias[j - 511]) masked, j-index = d + 511


def _build_program():
    nc = bacc.Bacc(None, target_bir_lowering=False)

    xT = nc.dram_tensor("xT", [C, T], FP16, kind="ExternalInput")
    wq = nc.dram_tensor("wq", [C, CL], FP16, kind="ExternalInput")
    wk = nc.dram_tensor("wk", [C, CL], FP16, kind="ExternalInput")
    wv = nc.dram_tensor("wv", [C, CL], FP16, kind="ExternalInput")
    wp = nc.dram_tensor("wp", [CL, C], FP16, kind="ExternalInput")
    bqk = nc.dram_tensor("bqk", [2, CL], FP32, kind="ExternalInput")
    bvr = nc.dram_tensor("bvr", [128, CL], FP32, kind="ExternalInput")
    wexp = nc.dram_tensor("wexp", [HL, EA_VEC], FP16, kind="ExternalInput")
    yp = nc.dram_tensor("yp", [C, T], FP32, kind="ExternalOutput")

    NT = T // 512  # 4 tq/t chunks of 512
    NK = T // 128  # 16 tk/t chunks of 128
    KC = C // 128  # 8 contraction chunks for qkv
    MC = CL // 128  # 4 m-chunks of local channels

    with tile.TileContext(nc) as tc:
        with (
            tc.tile_pool(name="w", bufs=1) as wpool,
            tc.tile_pool(name="big", bufs=1) as bigpool,
            tc.tile_pool(name="ea", bufs=2) as eapool,
            tc.tile_pool(name="tr", bufs=3) as tr,
            tc.tile_pool(name="ps", bufs=2, space="PSUM") as ps,
        ):
            # ---- weights / constants ----
            wq_sb = wpool.tile([128, KC, CL], FP16)
            wk_sb = wpool.tile([128, KC, CL], FP16)
            wv_sb = wpool.tile([128, KC, CL], FP16)
            wp_sb = wpool.tile([128, MC, C], FP16)
            nc.sync.dma_start(out=wq_sb, in_=wq.rearrange("(kc p) m -> p kc m", p=128))
            nc.sync.dma_start(out=wk_sb, in_=wk.rearrange("(kc p) m -> p kc m", p=128))
            nc.sync.dma_start(out=wv_sb, in_=wv.rearrange("(kc p) m -> p kc m", p=128))
            nc.sync.dma_start(out=wp_sb, in_=wp.rearrange("(kc p) m -> p kc m", p=128))

            bq_sb = wpool.tile([128, MC], FP32)
            bk_sb = wpool.tile([128, MC], FP32)
            bqk_r = bqk.rearrange("b (m p) -> b p m", p=128)
            nc.sync.dma_start(out=bq_sb, in_=bqk_r[0])
            nc.sync.dma_start(out=bk_sb, in_=bqk_r[1])

            # v-bias, broadcast across partitions on the host ([128, CL])
            bv_sb = wpool.tile([128, CL], FP32)
            nc.sync.dma_start(out=bv_sb, in_=bvr[:])

            ones16 = wpool.tile([128, 64], FP16)
            nc.vector.memset(ones16, 1.0)

            # ---- persistent activations ----
            qT_sb = bigpool.tile([128, MC, T], FP16)  # c' = m*128 + p
            kT_sb = bigpool.tile([128, MC, T], FP16)
            v_sb = bigpool.tile([128, NK, HL * 65], FP16)  # slot l: [v(64), ones]
            y_sb = bigpool.tile([128, MC, T], FP16)  # y_cat_T, c_in = m*128 + p

            for l in range(HL):
                nc.vector.memset(v_sb[:, :, l * 65 + 64 : l * 65 + 65], 1.0)

            # ---- stage 1: qkv projections ----
            xt_sb = bigpool.tile([128, KC, T], FP16)
            nc.sync.dma_start(out=xt_sb, in_=xT.rearrange("(kc p) t -> p kc t", p=128))

            for tch in range(NT):
                tsl = slice(tch * 512, (tch + 1) * 512)
                for m in range(MC):
                    msl = slice(m * 128, (m + 1) * 128)
                    pq = ps.tile([128, 512], FP32, tag="pq")
                    for kc in range(KC):
                        nc.tensor.matmul(
                            pq[:],
                            wq_sb[:, kc, msl],
                            xt_sb[:, kc, tsl],
                            start=(kc == 0),
                            stop=(kc == KC - 1),
                        )
                    nc.scalar.activation(
                        out=qT_sb[:, m, tsl], in_=pq[:],
                        func=mybir.ActivationFunctionType.Identity,
                        bias=bq_sb[:, m : m + 1], scale=1.0,
                    )
                    pk = ps.tile([128, 512], FP32, tag="pq")
                    for kc in range(KC):
                        nc.tensor.matmul(
                            pk[:],
                            wk_sb[:, kc, msl],
                            xt_sb[:, kc, tsl],
                            start=(kc == 0),
                            stop=(kc == KC - 1),
                        )
                    nc.scalar.activation(
                        out=kT_sb[:, m, tsl], in_=pk[:],
                        func=mybir.ActivationFunctionType.Identity,
                        bias=bk_sb[:, m : m + 1], scale=1.0,
                    )
                # v: plain layout [t, c'] so AV's lhsT has tk on partitions
                for ts in range(4):
                    t16 = tch * 4 + ts
                    pv = ps.tile([128, 512], FP32, tag="pq")
                    for kc in range(KC):
                        nc.tensor.matmul(
                            pv[:],
                            xt_sb[:, kc, t16 * 128 : (t16 + 1) * 128],
                            wv_sb[:, kc, :],
                            start=(kc == 0),
                            stop=(kc == KC - 1),
                        )
                    # scatter into 65-wide slots (even/odd strided copies) + bias
                    for par in range(2):
                        src = bass.AP(
                            tensor=pv.tensor, offset=pv.offset + par * 64,
                            ap=[pv.ap[0], [128, 4], [1, 64]],
                        )
                        srcb = bass.AP(
                            tensor=bv_sb.tensor, offset=bv_sb.offset + par * 64,
                            ap=[bv_sb.ap[0], [128, 4], [1, 64]],
                        )
                        base = v_sb[:, t16]
                        dst = bass.AP(
                            tensor=base.tensor, offset=base.offset + par * 65,
                            ap=[base.ap[0], [130, 4], [1, 64]],
                        )
                        nc.vector.tensor_add(out=dst, in0=src, in1=srcb)

            # ---- stage 2: attention per local head ----
            for l in range(HL):
                pb = (l % 2) * 64
                mq = l // 2
                # wexp rows are REVERSED on host: wr[j] = w[EA_VEC-1-j], so
                # A[p, x] = w[x - p] = wr[2431 + p - x]: positive partition step,
                # negative free step (walrus rejects negative partition steps).
                ea_sb = eapool.tile([128, EA_W], FP16, tag="ea")
                wexp_base = wexp[:]
                ea_src = bass.AP(
                    tensor=wexp_base.tensor,
                    offset=wexp_base.offset + l * EA_VEC + (EA_W - 1),
                    ap=[[1, 128], [-1, EA_W]],
                )
                nc.sync.dma_start(out=ea_sb, in_=ea_src)

                for c in range(NT):
                    qsl = slice(c * 512, (c + 1) * 512)
                    nj = 4 * c + 4
                    pav = ps.tile([65, 512], FP32, tag="pav")
                    for j in range(nj):
                        s_off = 512 * c - 128 * j + 384
                        pS = ps.tile([128, 512], FP32, tag="pS")
                        nc.tensor.matmul(
                            pS[:],
                            kT_sb[pb : pb + 64, mq, j * 128 : (j + 1) * 128],
                            qT_sb[pb : pb + 64, mq, qsl],
                            start=True,
                            stop=True,
                        )
                        p_sb = tr.tile([128, 512], FP16, tag="p")
                        nc.scalar.activation(
                            out=p_sb[:], in_=pS[:],
                            func=mybir.ActivationFunctionType.Exp,
                            scale=1.0 / math.sqrt(D),
                        )
                        pm_sb = tr.tile([128, 512], FP16, tag="pm")
                        nc.vector.tensor_mul(
                            out=pm_sb[:], in0=p_sb[:],
                            in1=ea_sb[:, s_off : s_off + 512],
                        )
                        nc.tensor.matmul(
                            pav[:],
                            v_sb[:, j, l * 65 : l * 65 + 65],
                            pm_sb[:],
                            start=(j == 0),
                            stop=(j == nj - 1),
                        )
                    # normalize: y = pav[0:64] * broadcast(1 / pav[64])
                    rec32 = tr.tile([128, 512], FP32, tag="rec32")
                    nc.vector.reciprocal(out=rec32[64:65, :], in_=pav[64:65, :])
                    rec16 = tr.tile([128, 512], FP16, tag="rec16")
                    nc.vector.tensor_copy(rec16[64:65, :], rec32[64:65, :])
                    bc_ps = ps.tile([64, 512], FP32, tag="bc")
                    nc.tensor.matmul(
                        bc_ps[:], ones16[64:65, :], rec16[64:65, :],
                        start=True, stop=True,
                    )
                    # DVE has one PSUM read port: bounce the broadcast to SBUF
                    bc_sb = tr.tile([64, 512], FP32, tag="bcsb")
                    nc.vector.tensor_copy(bc_sb[:], bc_ps[:])
                    if l % 2 == 0:
                        nc.vector.tensor_mul(
                            out=y_sb[0:64, mq, qsl], in0=pav[0:64, :], in1=bc_sb[:],
                        )
                    else:
                        ytmp = tr.tile([64, 512], FP16, tag="ytmp")
                        nc.vector.tensor_mul(out=ytmp[:], in0=pav[0:64, :], in1=bc_sb[:])
                        nc.sync.dma_start(out=y_sb[64:128, mq, qsl], in_=ytmp[:])

            # ---- stage 3: partial output projection ----
            for mo in range(C // 128):
                osl = slice(mo * 128, (mo + 1) * 128)
                for tch in range(NT):
                    tsl = slice(tch * 512, (tch + 1) * 512)
                    pp = ps.tile([128, 512], FP32, tag="pq")
                    for kc in range(MC):
                        nc.tensor.matmul(
                            pp[:],
                            wp_sb[:, kc, osl],
                            y_sb[:, kc, tsl],
                            start=(kc == 0),
                            stop=(kc == MC - 1),
                        )
                    yo_sb = tr.tile([128, 512], FP32, tag="yo")
                    nc.vector.tensor_copy(yo_sb[:], pp[:])
                    nc.sync.dma_start(out=yp[osl, tsl], in_=yo_sb[:])

    nc.compile()
    return nc


_NC = None
LAST_RESULTS = None


def _get_program():
    global _NC
    if _NC is None:
        _NC = _build_program()
    return _NC


def _rel_bias_buckets():
    """bucket(d) for d = q - k in [0, T): exact float32 replica of the reference."""
    d = np.arange(T)
    max_exact = NUM_BUCKETS // 2
    rpf = d.astype(np.float32) / np.float32(max_exact) + np.float32(1e-10)
    val = (
        np.log(rpf)
        / np.float32(math.log(MAX_DISTANCE / max_exact))
        * np.float32(NUM_BUCKETS - max_exact)
    )
    large = max_exact + val.astype(np.int32)
    large = np.minimum(large, NUM_BUCKETS - 1)
    return np.where(d < max_exact, d, large)


def _make_in_maps(x, W_attn, b_attn, W_proj, rel_emb):
    buckets = _rel_bias_buckets()  # [T]
    bias_by_dist = rel_emb[buckets, :]  # [T, H] fp32
    # wexp[h, j] = exp(bias[j - 511]) for j >= 511 else 0   (j - 511 = distance d)
    wexp_all = np.zeros((H, EA_VEC), dtype=np.float32)
    wexp_all[:, 511 : 511 + T] = np.exp(bias_by_dist.T)
    wexp_all = wexp_all[:, ::-1].astype(np.float16)  # reversed: see ea_src in _build_program

    in_maps = []
    for core in range(NCORES):
        b, hg = core // 2, core % 2
        csl = slice(hg * CL, (hg + 1) * CL)
        in_maps.append(
            {
                "xT": np.ascontiguousarray(x[b].T).astype(np.float16),
                "wq": np.ascontiguousarray(W_attn[csl, :].T).astype(np.float16),
                "wk": np.ascontiguousarray(W_attn[C + hg * CL : C + (hg + 1) * CL, :].T).astype(np.float16),
                "wv": np.ascontiguousarray(W_attn[2 * C + hg * CL : 2 * C + (hg + 1) * CL, :].T).astype(np.float16),
                "wp": np.ascontiguousarray(W_proj[:, csl].T).astype(np.float16),
                "bqk": np.stack(
                    [b_attn[csl], b_attn[C + hg * CL : C + (hg + 1) * CL]]
                ).astype(np.float32),
                "bvr": np.ascontiguousarray(np.broadcast_to(
                    b_attn[2 * C + hg * CL : 2 * C + (hg + 1) * CL].astype(np.float32), (128, CL)
                )),
                "wexp": np.ascontiguousarray(wexp_all[hg * HL : (hg + 1) * HL]),
            }
        )
    return in_maps


def kernel(x, W_attn, b_attn, W_proj, b_proj, rel_emb):
    x = np.asarray(x)
    W_attn = np.asarray(W_attn)
    b_attn = np.asarray(b_attn)
    W_proj = np.asarray(W_proj)
    b_proj = np.asarray(b_proj)
    rel_emb = np.asarray(rel_emb)

    in_maps = _make_in_maps(x, W_attn, b_attn, W_proj, rel_emb)
    nc = _get_program()
    res = bass_utils.run_bass_kernel_spmd(nc, in_maps, core_ids=list(range(NCORES)))
    global LAST_RESULTS
    LAST_RESULTS = res

    y = np.empty((B, T, C), dtype=np.float32)
    for b in range(B):
        ypT = res.results[2 * b]["yp"] + res.results[2 * b + 1]["yp"]
        y[b] = ypT.T + b_proj[None, :].astype(np.float32)
    return y


# revision 16
# speedup vs baseline: 5.1190x; 5.1190x over previous
"""Trainium2 Bass kernel for causal self-attention with T5 relative position bias.

Problem (hardcoded): B=4, T=2048, C=1024, H=16, D=64, NUM_BUCKETS=32, MAX_DISTANCE=128.
Sharding over 8 cores: core c -> (batch b=c//2, head-group hg=c%2 of 8 heads).
Each core computes qkv projection for its heads, causal attention, and a partial
output projection (its heads' rows of W_proj); host sums the two partials per batch.

On-chip layout notes:
  - x, q, k are kept transposed ([C, T]-style, channel on partitions) so every
    matmul contracts over the partition dim with no on-chip transposes.
  - Attention logits are computed transposed: S_T[tk, tq] = k_h^T q_h (K=64).
  - Softmax skips max-subtraction (logits ~ N(0,1); exp <= e^7 fits fp16 easily).
  - The T5 bias + causal mask are folded into one fp16 Toeplitz table per head:
    expAm[p, x] = exp(bias[d]) * (d >= 0), d = x - p - 384.  P = exp(S/8) * expAm.
    The [128, 2432] table is expanded on-device from a 2559-float vector by a
    DMA whose DRAM-side access pattern has a -1 partition step.
  - Row sums come free from a ones-column appended to V (AV matmul M=65);
    normalization multiplies by a PE-broadcast reciprocal row.
"""

import sys

sys.path.insert(0, "/opt/trn_rl_repo")

import math

import numpy as np

import concourse.bacc as bacc
import concourse.bass as bass
import concourse.mybir as mybir
import concourse.tile as tile
from concourse import bass_utils

B, T, C = 4, 2048, 1024
H, D = 16, 64
NUM_BUCKETS, MAX_DISTANCE = 32, 128
HL = 8  # local heads per core
CL = HL * D  # 512 local channels
NCORES = 8

FP16 = mybir.dt.float16
FP32 = mybir.dt.float32

# expAm table geometry: slice start s = (tq0 - tk0) + 384 in [0, 1920], width 512
EA_W = 2432  # 1920 + 512
EA_VEC = EA_W + 127  # 2559: w[j] = exp(bias[j - 511]) masked, j-index = d + 511


def _build_program():
    nc = bacc.Bacc(None, target_bir_lowering=False)

    xT = nc.dram_tensor("xT", [C, T], FP16, kind="ExternalInput")
    wq = nc.dram_tensor("wq", [C, CL], FP16, kind="ExternalInput")
    wk = nc.dram_tensor("wk", [C, CL], FP16, kind="ExternalInput")
    wv = nc.dram_tensor("wv", [C, CL], FP16, kind="ExternalInput")
    wp = nc.dram_tensor("wp", [CL, C], FP16, kind="ExternalInput")
    bqk = nc.dram_tensor("bqk", [2, CL], FP32, kind="ExternalInput")
    bvr = nc.dram_tensor("bvr", [128, CL], FP32, kind="ExternalInput")
    wexp = nc.dram_tensor("wexp", [HL, 128, EA_W], FP16, kind="ExternalInput")
    yp = nc.dram_tensor("yp", [C, T], FP32, kind="ExternalOutput")

    NT = T // 512  # 4 tq/t chunks of 512
    NK = T // 128  # 16 tk/t chunks of 128
    KC = C // 128  # 8 contraction chunks for qkv
    MC = CL // 128  # 4 m-chunks of local channels

    with tile.TileContext(nc) as tc:
        with (
            tc.tile_pool(name="w", bufs=1) as wpool,
            tc.tile_pool(name="big", bufs=1) as bigpool,
            tc.tile_pool(name="ea", bufs=2) as eapool,
            tc.tile_pool(name="tr", bufs=3) as tr,
            tc.tile_pool(name="ps", bufs=2, space="PSUM") as ps,
        ):
            # ---- weights / constants ----
            wq_sb = wpool.tile([128, KC, CL], FP16)
            wk_sb = wpool.tile([128, KC, CL], FP16)
            wv_sb = wpool.tile([128, KC, CL], FP16)
            wp_sb = wpool.tile([128, MC, C], FP16)
            nc.sync.dma_start(out=wq_sb, in_=wq.rearrange("(kc p) m -> p kc m", p=128))
            nc.sync.dma_start(out=wk_sb, in_=wk.rearrange("(kc p) m -> p kc m", p=128))
            nc.sync.dma_start(out=wv_sb, in_=wv.rearrange("(kc p) m -> p kc m", p=128))
            nc.sync.dma_start(out=wp_sb, in_=wp.rearrange("(kc p) m -> p kc m", p=128))

            bq_sb = wpool.tile([128, MC], FP32)
            bk_sb = wpool.tile([128, MC], FP32)
            bqk_r = bqk.rearrange("b (m p) -> b p m", p=128)
            nc.sync.dma_start(out=bq_sb, in_=bqk_r[0])
            nc.sync.dma_start(out=bk_sb, in_=bqk_r[1])

            # v-bias, broadcast across partitions on the host ([128, CL])
            bv_sb = wpool.tile([128, CL], FP32)
            nc.sync.dma_start(out=bv_sb, in_=bvr[:])

            ones16 = wpool.tile([128, 64], FP16)
            nc.vector.memset(ones16, 1.0)

            # ---- persistent activations ----
            qT_sb = bigpool.tile([128, MC, T], FP16)  # c' = m*128 + p
            kT_sb = bigpool.tile([128, MC, T], FP16)
            v_sb = bigpool.tile([128, NK, HL * 65], FP16)  # slot l: [v(64), ones]
            y_sb = bigpool.tile([128, MC, T], FP16)  # y_cat_T, c_in = m*128 + p

            for l in range(HL):
                nc.vector.memset(v_sb[:, :, l * 65 + 64 : l * 65 + 65], 1.0)

            # ---- stage 1: qkv projections ----
            xt_sb = bigpool.tile([128, KC, T], FP16)
            nc.sync.dma_start(out=xt_sb, in_=xT.rearrange("(kc p) t -> p kc t", p=128))

            for tch in range(NT):
                tsl = slice(tch * 512, (tch + 1) * 512)
                for m in range(MC):
                    msl = slice(m * 128, (m + 1) * 128)
                    pq = ps.tile([128, 512], FP32, tag="pq")
                    for kc in range(KC):
                        nc.tensor.matmul(
                            pq[:],
                            wq_sb[:, kc, msl],
                            xt_sb[:, kc, tsl],
                            start=(kc == 0),
                            stop=(kc == KC - 1),
                        )
                    nc.scalar.activation(
                        out=qT_sb[:, m, tsl], in_=pq[:],
                        func=mybir.ActivationFunctionType.Identity,
                        bias=bq_sb[:, m : m + 1], scale=1.0,
                    )
                    pk = ps.tile([128, 512], FP32, tag="pq")
                    for kc in range(KC):
                        nc.tensor.matmul(
                            pk[:],
                            wk_sb[:, kc, msl],
                            xt_sb[:, kc, tsl],
                            start=(kc == 0),
                            stop=(kc == KC - 1),
                        )
                    nc.scalar.activation(
                        out=kT_sb[:, m, tsl], in_=pk[:],
                        func=mybir.ActivationFunctionType.Identity,
                        bias=bk_sb[:, m : m + 1], scale=1.0,
                    )
                # v: plain layout [t, c'] so AV's lhsT has tk on partitions
                for ts in range(4):
                    t16 = tch * 4 + ts
                    pv = ps.tile([128, 512], FP32, tag="pq")
                    for kc in range(KC):
                        nc.tensor.matmul(
                            pv[:],
                            xt_sb[:, kc, t16 * 128 : (t16 + 1) * 128],
                            wv_sb[:, kc, :],
                            start=(kc == 0),
                            stop=(kc == KC - 1),
                        )
                    # scatter into 65-wide slots (even/odd strided copies) + bias
                    for par in range(2):
                        src = bass.AP(
                            tensor=pv.tensor, offset=pv.offset + par * 64,
                            ap=[pv.ap[0], [128, 4], [1, 64]],
                        )
                        srcb = bass.AP(
                            tensor=bv_sb.tensor, offset=bv_sb.offset + par * 64,
                            ap=[bv_sb.ap[0], [128, 4], [1, 64]],
                        )
                        base = v_sb[:, t16]
                        dst = bass.AP(
                            tensor=base.tensor, offset=base.offset + par * 65,
                            ap=[base.ap[0], [130, 4], [1, 64]],
                        )
                        nc.vector.tensor_add(out=dst, in0=src, in1=srcb)

            # ---- stage 2: attention per local head ----
            for l in range(HL):
                pb = (l % 2) * 64
                mq = l // 2
                # host-expanded Toeplitz table (strided/reversed DMA reads decompose
                # into per-element descriptors and take ~300us; a plain contiguous
                # 600KB DMA takes ~2us)
                ea_sb = eapool.tile([128, EA_W], FP16, tag="ea")
                nc.sync.dma_start(out=ea_sb, in_=wexp[l])

                for c in range(NT):
                    qsl = slice(c * 512, (c + 1) * 512)
                    nj = 4 * c + 4
                    pav = ps.tile([65, 512], FP32, tag="pav")
                    for j in range(nj):
                        s_off = 512 * c - 128 * j + 384
                        pS = ps.tile([128, 512], FP32, tag="pS")
                        nc.tensor.matmul(
                            pS[:],
                            kT_sb[pb : pb + 64, mq, j * 128 : (j + 1) * 128],
                            qT_sb[pb : pb + 64, mq, qsl],
                            start=True,
                            stop=True,
                        )
                        p_sb = tr.tile([128, 512], FP16, tag="p")
                        nc.scalar.activation(
                            out=p_sb[:], in_=pS[:],
                            func=mybir.ActivationFunctionType.Exp,
                            scale=1.0 / math.sqrt(D),
                        )
                        pm_sb = tr.tile([128, 512], FP16, tag="pm")
                        nc.vector.tensor_mul(
                            out=pm_sb[:], in0=p_sb[:],
                            in1=ea_sb[:, s_off : s_off + 512],
                        )
                        nc.tensor.matmul(
                            pav[:],
                            v_sb[:, j, l * 65 : l * 65 + 65],
                            pm_sb[:],
                            start=(j == 0),
                            stop=(j == nj - 1),
                        )
                    # normalize: y = pav[0:64] * broadcast(1 / pav[64])
                    rec32 = tr.tile([128, 512], FP32, tag="rec32")
                    nc.vector.reciprocal(out=rec32[64:65, :], in_=pav[64:65, :])
                    rec16 = tr.tile([128, 512], FP16, tag="rec16")
                    nc.vector.tensor_copy(rec16[64:65, :], rec32[64:65, :])
                    bc_ps = ps.tile([64, 512], FP32, tag="bc")
                    nc.tensor.matmul(
                        bc_ps[:], ones16[64:65, :], rec16[64:65, :],
                        start=True, stop=True,
                    )
                    # DVE has one PSUM read port: bounce the broadcast to SBUF
                    bc_sb = tr.tile([64, 512], FP32, tag="bcsb")
                    nc.vector.tensor_copy(bc_sb[:], bc_ps[:])
                    if l % 2 == 0:
                        nc.vector.tensor_mul(
                            out=y_sb[0:64, mq, qsl], in0=pav[0:64, :], in1=bc_sb[:],
                        )
                    else:
                        ytmp = tr.tile([64, 512], FP16, tag="ytmp")
                        nc.vector.tensor_mul(out=ytmp[:], in0=pav[0:64, :], in1=bc_sb[:])
                        nc.sync.dma_start(out=y_sb[64:128, mq, qsl], in_=ytmp[:])

            # ---- stage 3: partial output projection ----
            for mo in range(C // 128):
                osl = slice(mo * 128, (mo + 1) * 128)
                for tch in range(NT):
                    tsl = slice(tch * 512, (tch + 1) * 512)
                    pp = ps.tile([128, 512], FP32, tag="pq")
                    for kc in range(MC):
                        nc.tensor.matmul(
                            pp[:],
                            wp_sb[:, kc, osl],
                            y_sb[:, kc, tsl],
                            start=(kc == 0),
                            stop=(kc == MC - 1),
                        )
                    yo_sb = tr.tile([128, 512], FP32, tag="yo")
                    nc.vector.tensor_copy(yo_sb[:], pp[:])
                    nc.sync.dma_start(out=yp[osl, tsl], in_=yo_sb[:])

    nc.compile()
    return nc


_NC = None
LAST_RESULTS = None


def _get_program():
    global _NC
    if _NC is None:
        _NC = _build_program()
    return _NC


def _rel_bias_buckets():
    """bucket(d) for d = q - k in [0, T): exact float32 replica of the reference."""
    d = np.arange(T)
    max_exact = NUM_BUCKETS // 2
    rpf = d.astype(np.float32) / np.float32(max_exact) + np.float32(1e-10)
    val = (
        np.log(rpf)
        / np.float32(math.log(MAX_DISTANCE / max_exact))
        * np.float32(NUM_BUCKETS - max_exact)
    )
    large = max_exact + val.astype(np.int32)
    large = np.minimum(large, NUM_BUCKETS - 1)
    return np.where(d < max_exact, d, large)


def _make_in_maps(x, W_attn, b_attn, W_proj, rel_emb):
    buckets = _rel_bias_buckets()  # [T]
    bias_by_dist = rel_emb[buckets, :]  # [T, H] fp32
    # vec[h, j] = exp(bias[j - 511]) for j >= 511 else 0   (j - 511 = distance d)
    vec = np.zeros((H, EA_VEC), dtype=np.float32)
    vec[:, 511 : 511 + T] = np.exp(bias_by_dist.T)
    vec = vec.astype(np.float16)
    # expand to the per-head Toeplitz table A[h, p, x] = vec[h, x - p + 127]
    sw = np.lib.stride_tricks.sliding_window_view(vec, EA_W, axis=1)  # [H, 128, EA_W]
    wexp_all = np.ascontiguousarray(sw[:, ::-1, :])

    in_maps = []
    for core in range(NCORES):
        b, hg = core // 2, core % 2
        csl = slice(hg * CL, (hg + 1) * CL)
        in_maps.append(
            {
                "xT": np.ascontiguousarray(x[b].T).astype(np.float16),
                "wq": np.ascontiguousarray(W_attn[csl, :].T).astype(np.float16),
                "wk": np.ascontiguousarray(W_attn[C + hg * CL : C + (hg + 1) * CL, :].T).astype(np.float16),
                "wv": np.ascontiguousarray(W_attn[2 * C + hg * CL : 2 * C + (hg + 1) * CL, :].T).astype(np.float16),
                "wp": np.ascontiguousarray(W_proj[:, csl].T).astype(np.float16),
                "bqk": np.stack(
                    [b_attn[csl], b_attn[C + hg * CL : C + (hg + 1) * CL]]
                ).astype(np.float32),
                "bvr": np.ascontiguousarray(np.broadcast_to(
                    b_attn[2 * C + hg * CL : 2 * C + (hg + 1) * CL].astype(np.float32), (128, CL)
                )),
                "wexp": np.ascontiguousarray(wexp_all[hg * HL : (hg + 1) * HL]),
            }
        )
    return in_maps


def kernel(x, W_attn, b_attn, W_proj, b_proj, rel_emb):
    x = np.asarray(x)
    W_attn = np.asarray(W_attn)
    b_attn = np.asarray(b_attn)
    W_proj = np.asarray(W_proj)
    b_proj = np.asarray(b_proj)
    rel_emb = np.asarray(rel_emb)

    in_maps = _make_in_maps(x, W_attn, b_attn, W_proj, rel_emb)
    nc = _get_program()
    res = bass_utils.run_bass_kernel_spmd(nc, in_maps, core_ids=list(range(NCORES)))
    global LAST_RESULTS
    LAST_RESULTS = res

    y = np.empty((B, T, C), dtype=np.float32)
    for b in range(B):
        ypT = res.results[2 * b]["yp"] + res.results[2 * b + 1]["yp"]
        y[b] = ypT.T + b_proj[None, :].astype(np.float32)
    return y


# revision 21
# speedup vs baseline: 5.2169x; 1.0191x over previous
"""Trainium2 Bass kernel for causal self-attention with T5 relative position bias.

Problem (hardcoded): B=4, T=2048, C=1024, H=16, D=64, NUM_BUCKETS=32, MAX_DISTANCE=128.
Sharding over 8 cores: core c -> (batch b=c//2, head-group hg=c%2 of 8 heads).
Each core computes qkv projection for its heads, causal attention, and a partial
output projection (its heads' rows of W_proj); host sums the two partials per batch.

On-chip layout notes:
  - x, q, k are kept transposed ([C, T]-style, channel on partitions) so every
    matmul contracts over the partition dim with no on-chip transposes.
  - Attention logits are computed transposed: S_T[tk, tq] = k_h^T q_h (K=64).
  - Softmax skips max-subtraction (logits ~ N(0,1); exp <= e^7 fits fp16 easily).
  - The T5 bias + causal mask are folded into one fp16 Toeplitz table per head:
    expAm[p, x] = exp(bias[d]) * (d >= 0), d = x - p - 384.  P = exp(S/8) * expAm.
    The [128, 2432] table is expanded on-device from a 2559-float vector by a
    DMA whose DRAM-side access pattern has a -1 partition step.
  - Row sums come free from a ones-column appended to V (AV matmul M=65);
    normalization multiplies by a PE-broadcast reciprocal row.
"""

import sys

sys.path.insert(0, "/opt/trn_rl_repo")

import math

import numpy as np

import concourse.bacc as bacc
import concourse.bass as bass
import concourse.mybir as mybir
import concourse.tile as tile
from concourse import bass_utils

B, T, C = 4, 2048, 1024
H, D = 16, 64
NUM_BUCKETS, MAX_DISTANCE = 32, 128
HL = 8  # local heads per core
CL = HL * D  # 512 local channels
NCORES = 8

FP16 = mybir.dt.float16
FP32 = mybir.dt.float32

# expAm table geometry: slice start s = (tq0 - tk0) + 384 in [0, 1920], width 512
EA_W = 2432  # 1920 + 512
EA_VEC = EA_W + 127  # 2559: w[j] = exp(bias[j - 511]) masked, j-index = d + 511


def _build_program():
    nc = bacc.Bacc(None, target_bir_lowering=False)

    xT = nc.dram_tensor("xT", [C, T], FP16, kind="ExternalInput")
    wq = nc.dram_tensor("wq", [C, CL], FP16, kind="ExternalInput")
    wk = nc.dram_tensor("wk", [C, CL], FP16, kind="ExternalInput")
    wv = nc.dram_tensor("wv", [C, CL], FP16, kind="ExternalInput")
    wp = nc.dram_tensor("wp", [CL, C], FP16, kind="ExternalInput")
    bqk = nc.dram_tensor("bqk", [2, CL], FP32, kind="ExternalInput")
    bvr = nc.dram_tensor("bvr", [128, CL], FP32, kind="ExternalInput")
    wexp = nc.dram_tensor("wexp", [HL, 128, EA_W], FP16, kind="ExternalInput")
    yp = nc.dram_tensor("yp", [C, T], FP32, kind="ExternalOutput")

    NT = T // 512  # 4 tq/t chunks of 512
    NK = T // 128  # 16 tk/t chunks of 128
    KC = C // 128  # 8 contraction chunks for qkv
    MC = CL // 128  # 4 m-chunks of local channels

    with tile.TileContext(nc) as tc:
        with (
            tc.tile_pool(name="w", bufs=1) as wpool,
            tc.tile_pool(name="big", bufs=1) as bigpool,
            tc.tile_pool(name="ea", bufs=2) as eapool,
            tc.tile_pool(name="tr", bufs=3) as tr,
            tc.tile_pool(name="ps", bufs=2, space="PSUM") as ps,
        ):
            # ---- weights / constants ----
            wq_sb = wpool.tile([128, KC, CL], FP16)
            wk_sb = wpool.tile([128, KC, CL], FP16)
            wv_sb = wpool.tile([128, KC, CL], FP16)
            wp_sb = wpool.tile([128, MC, C], FP16)
            nc.sync.dma_start(out=wq_sb, in_=wq.rearrange("(kc p) m -> p kc m", p=128))
            nc.sync.dma_start(out=wk_sb, in_=wk.rearrange("(kc p) m -> p kc m", p=128))
            nc.sync.dma_start(out=wv_sb, in_=wv.rearrange("(kc p) m -> p kc m", p=128))
            nc.sync.dma_start(out=wp_sb, in_=wp.rearrange("(kc p) m -> p kc m", p=128))

            bq_sb = wpool.tile([128, MC], FP32)
            bk_sb = wpool.tile([128, MC], FP32)
            bqk_r = bqk.rearrange("b (m p) -> b p m", p=128)
            nc.sync.dma_start(out=bq_sb, in_=bqk_r[0])
            nc.sync.dma_start(out=bk_sb, in_=bqk_r[1])

            # v-bias, broadcast across partitions on the host ([128, CL])
            bv_sb = wpool.tile([128, CL], FP32)
            nc.sync.dma_start(out=bv_sb, in_=bvr[:])

            ones16 = wpool.tile([128, 64], FP16)
            nc.vector.memset(ones16, 1.0)

            # ---- persistent activations ----
            qT_sb = bigpool.tile([128, MC, T], FP16)  # c' = m*128 + p
            kT_sb = bigpool.tile([128, MC, T], FP16)
            v_sb = bigpool.tile([128, NK, HL * 65], FP16)  # slot l: [v(64), ones]
            y_sb = bigpool.tile([128, MC, T], FP16)  # y_cat_T, c_in = m*128 + p

            for l in range(HL):
                nc.vector.memset(v_sb[:, :, l * 65 + 64 : l * 65 + 65], 1.0)

            # ---- stage 1: qkv projections ----
            xt_sb = bigpool.tile([128, KC, T], FP16)
            nc.sync.dma_start(out=xt_sb, in_=xT.rearrange("(kc p) t -> p kc t", p=128))

            for tch in range(NT):
                tsl = slice(tch * 512, (tch + 1) * 512)
                for m in range(MC):
                    msl = slice(m * 128, (m + 1) * 128)
                    pq = ps.tile([128, 512], FP32, tag="pq")
                    for kc in range(KC):
                        nc.tensor.matmul(
                            pq[:],
                            wq_sb[:, kc, msl],
                            xt_sb[:, kc, tsl],
                            start=(kc == 0),
                            stop=(kc == KC - 1),
                        )
                    nc.scalar.activation(
                        out=qT_sb[:, m, tsl], in_=pq[:],
                        func=mybir.ActivationFunctionType.Identity,
                        bias=bq_sb[:, m : m + 1], scale=1.0,
                    )
                    pk = ps.tile([128, 512], FP32, tag="pq")
                    for kc in range(KC):
                        nc.tensor.matmul(
                            pk[:],
                            wk_sb[:, kc, msl],
                            xt_sb[:, kc, tsl],
                            start=(kc == 0),
                            stop=(kc == KC - 1),
                        )
                    nc.scalar.activation(
                        out=kT_sb[:, m, tsl], in_=pk[:],
                        func=mybir.ActivationFunctionType.Identity,
                        bias=bk_sb[:, m : m + 1], scale=1.0,
                    )
                # v: plain layout [t, c'] so AV's lhsT has tk on partitions
                for ts in range(4):
                    t16 = tch * 4 + ts
                    pv = ps.tile([128, 512], FP32, tag="pq")
                    for kc in range(KC):
                        nc.tensor.matmul(
                            pv[:],
                            xt_sb[:, kc, t16 * 128 : (t16 + 1) * 128],
                            wv_sb[:, kc, :],
                            start=(kc == 0),
                            stop=(kc == KC - 1),
                        )
                    # scatter into 65-wide slots (even/odd strided copies) + bias
                    for par in range(2):
                        src = bass.AP(
                            tensor=pv.tensor, offset=pv.offset + par * 64,
                            ap=[pv.ap[0], [128, 4], [1, 64]],
                        )
                        srcb = bass.AP(
                            tensor=bv_sb.tensor, offset=bv_sb.offset + par * 64,
                            ap=[bv_sb.ap[0], [128, 4], [1, 64]],
                        )
                        base = v_sb[:, t16]
                        dst = bass.AP(
                            tensor=base.tensor, offset=base.offset + par * 65,
                            ap=[base.ap[0], [130, 4], [1, 64]],
                        )
                        nc.vector.tensor_add(out=dst, in0=src, in1=srcb)

            # ---- stage 2: attention per local head ----
            for l in range(HL):
                pb = (l % 2) * 64
                mq = l // 2
                # host-expanded Toeplitz table (strided/reversed DMA reads decompose
                # into per-element descriptors and take ~300us; a plain contiguous
                # 600KB DMA takes ~2us)
                ea_sb = eapool.tile([128, EA_W], FP16, tag="ea")
                nc.sync.dma_start(out=ea_sb, in_=wexp[l])

                for c in range(NT):
                    nj = 4 * c + 4
                    pav = ps.tile([65, 512], FP32, tag="pav")
                    for j in range(nj):
                        # columns below the causal diagonal are fully masked:
                        # compute S/exp/mult only for tq >= tk; GpSimd (idle)
                        # zeroes the masked strip so the full-width AV matmul
                        # reads a fully-written tile
                        off = max(0, 128 * j - 512 * c)
                        csl = slice(off, 512)
                        qsl = slice(c * 512 + off, (c + 1) * 512)
                        s_off = 512 * c - 128 * j + 384 + off
                        pS = ps.tile([128, 512], FP32, tag="pS")
                        nc.tensor.matmul(
                            pS[:, csl],
                            kT_sb[pb : pb + 64, mq, j * 128 : (j + 1) * 128],
                            qT_sb[pb : pb + 64, mq, qsl],
                            start=True,
                            stop=True,
                        )
                        p_sb = tr.tile([128, 512], FP16, tag="p")
                        nc.scalar.activation(
                            out=p_sb[:, csl], in_=pS[:, csl],
                            func=mybir.ActivationFunctionType.Exp,
                            scale=1.0 / math.sqrt(D),
                        )
                        pm_sb = tr.tile([128, 512], FP16, tag="pm")
                        if off:
                            nc.gpsimd.memset(pm_sb[:, 0:off], 0.0)
                        nc.vector.tensor_mul(
                            out=pm_sb[:, csl], in0=p_sb[:, csl],
                            in1=ea_sb[:, s_off : s_off + 512 - off],
                        )
                        nc.tensor.matmul(
                            pav[:],
                            v_sb[:, j, l * 65 : l * 65 + 65],
                            pm_sb[:],
                            start=(j == 0),
                            stop=(j == nj - 1),
                        )
                    # normalize: y = pav[0:64] * broadcast(1 / pav[64])
                    rec32 = tr.tile([128, 512], FP32, tag="rec32")
                    nc.vector.reciprocal(out=rec32[64:65, :], in_=pav[64:65, :])
                    rec16 = tr.tile([128, 512], FP16, tag="rec16")
                    nc.vector.tensor_copy(rec16[64:65, :], rec32[64:65, :])
                    bc_ps = ps.tile([64, 512], FP32, tag="bc")
                    nc.tensor.matmul(
                        bc_ps[:], ones16[64:65, :], rec16[64:65, :],
                        start=True, stop=True,
                    )
                    # DVE has one PSUM read port: bounce the broadcast to SBUF
                    bc_sb = tr.tile([64, 512], FP32, tag="bcsb")
                    nc.vector.tensor_copy(bc_sb[:], bc_ps[:])
                    fullq = slice(c * 512, (c + 1) * 512)
                    if l % 2 == 0:
                        nc.vector.tensor_mul(
                            out=y_sb[0:64, mq, fullq], in0=pav[0:64, :], in1=bc_sb[:],
                        )
                    else:
                        ytmp = tr.tile([64, 512], FP16, tag="ytmp")
                        nc.vector.tensor_mul(out=ytmp[:], in0=pav[0:64, :], in1=bc_sb[:])
                        nc.sync.dma_start(out=y_sb[64:128, mq, fullq], in_=ytmp[:])

            # ---- stage 3: partial output projection ----
            for mo in range(C // 128):
                osl = slice(mo * 128, (mo + 1) * 128)
                for tch in range(NT):
                    tsl = slice(tch * 512, (tch + 1) * 512)
                    pp = ps.tile([128, 512], FP32, tag="pq")
                    for kc in range(MC):
                        nc.tensor.matmul(
                            pp[:],
                            wp_sb[:, kc, osl],
                            y_sb[:, kc, tsl],
                            start=(kc == 0),
                            stop=(kc == MC - 1),
                        )
                    yo_sb = tr.tile([128, 512], FP32, tag="yo")
                    nc.vector.tensor_copy(yo_sb[:], pp[:])
                    nc.sync.dma_start(out=yp[osl, tsl], in_=yo_sb[:])

    nc.compile()
    return nc


_NC = None
LAST_RESULTS = None


def _get_program():
    global _NC
    if _NC is None:
        _NC = _build_program()
    return _NC


def _rel_bias_buckets():
    """bucket(d) for d = q - k in [0, T): exact float32 replica of the reference."""
    d = np.arange(T)
    max_exact = NUM_BUCKETS // 2
    rpf = d.astype(np.float32) / np.float32(max_exact) + np.float32(1e-10)
    val = (
        np.log(rpf)
        / np.float32(math.log(MAX_DISTANCE / max_exact))
        * np.float32(NUM_BUCKETS - max_exact)
    )
    large = max_exact + val.astype(np.int32)
    large = np.minimum(large, NUM_BUCKETS - 1)
    return np.where(d < max_exact, d, large)


def _make_in_maps(x, W_attn, b_attn, W_proj, rel_emb):
    buckets = _rel_bias_buckets()  # [T]
    bias_by_dist = rel_emb[buckets, :]  # [T, H] fp32
    # vec[h, j] = exp(bias[j - 511]) for j >= 511 else 0   (j - 511 = distance d)
    vec = np.zeros((H, EA_VEC), dtype=np.float32)
    vec[:, 511 : 511 + T] = np.exp(bias_by_dist.T)
    vec = vec.astype(np.float16)
    # expand to the per-head Toeplitz table A[h, p, x] = vec[h, x - p + 127]
    sw = np.lib.stride_tricks.sliding_window_view(vec, EA_W, axis=1)  # [H, 128, EA_W]
    wexp_all = np.ascontiguousarray(sw[:, ::-1, :])

    in_maps = []
    for core in range(NCORES):
        b, hg = core // 2, core % 2
        csl = slice(hg * CL, (hg + 1) * CL)
        in_maps.append(
            {
                "xT": np.ascontiguousarray(x[b].T).astype(np.float16),
                "wq": np.ascontiguousarray(W_attn[csl, :].T).astype(np.float16),
                "wk": np.ascontiguousarray(W_attn[C + hg * CL : C + (hg + 1) * CL, :].T).astype(np.float16),
                "wv": np.ascontiguousarray(W_attn[2 * C + hg * CL : 2 * C + (hg + 1) * CL, :].T).astype(np.float16),
                "wp": np.ascontiguousarray(W_proj[:, csl].T).astype(np.float16),
                "bqk": np.stack(
                    [b_attn[csl], b_attn[C + hg * CL : C + (hg + 1) * CL]]
                ).astype(np.float32),
                "bvr": np.ascontiguousarray(np.broadcast_to(
                    b_attn[2 * C + hg * CL : 2 * C + (hg + 1) * CL].astype(np.float32), (128, CL)
                )),
                "wexp": np.ascontiguousarray(wexp_all[hg * HL : (hg + 1) * HL]),
            }
        )
    return in_maps


def kernel(x, W_attn, b_attn, W_proj, b_proj, rel_emb):
    x = np.asarray(x)
    W_attn = np.asarray(W_attn)
    b_attn = np.asarray(b_attn)
    W_proj = np.asarray(W_proj)
    b_proj = np.asarray(b_proj)
    rel_emb = np.asarray(rel_emb)

    in_maps = _make_in_maps(x, W_attn, b_attn, W_proj, rel_emb)
    nc = _get_program()
    res = bass_utils.run_bass_kernel_spmd(nc, in_maps, core_ids=list(range(NCORES)))
    global LAST_RESULTS
    LAST_RESULTS = res

    y = np.empty((B, T, C), dtype=np.float32)
    for b in range(B):
        ypT = res.results[2 * b]["yp"] + res.results[2 * b + 1]["yp"]
        y[b] = ypT.T + b_proj[None, :].astype(np.float32)
    return y
